# revision 1
# baseline (speedup 1.0000x reference)
"""CoSen cross-entropy loss kernel for Trainium2 (8 NeuronCores, data-parallel).

Math note: the reference computes
    m_i   = xi[label_i, argmax_j x_ij]
    denom = log(sum_j m_i * exp(x_ij)) = log(m_i) + logsumexp(x_i)
    log_s = log(m_i) + x - denom = x - logsumexp(x_i)
so m (and therefore xi and the argmax) cancels exactly and the loss is plain
cross-entropy:  nll = mean_i( logsumexp(x_i) - x[i, label_i] ).

A second invariance: logsumexp is invariant to permuting a row's columns, so
the host-side sharding swaps each row's label column into column 0. The label
term is then just column 0 of every row -- a free strided read on device --
and no gather/one-hot machinery is needed at all.

Device strategy (per core, 4096 rows x 1000 cols):
  - host casts scores to fp8e4m3 (quarters HBM traffic vs fp32; the e4m3
    rounding is mean-zero; measured end-to-end rel err ~1.6e-3 vs 2e-2 gate)
  - stream x in [128, 16, 1000] fp8 group tiles (1 MB HWDGE DMAs)
  - per 128-row block, exp + row-sum runs on one of three engines (pattern):
      'A' ScalarE table exp (exact, fp16 out) + fused accum
      'D' VectorE Schraudolph int16-bitcast exp, then a 2nd VectorE
          tensor_scalar pass whose accum_out is the row-sum (4x mode)
      'P' GpSimd Schraudolph convert, VectorE accum pass
  - per group, one tiny tensor_scalar converts column 0 (strided [128,16]
    fp8) into the per-block label values
  - tail: bitcast-ln of the row sums, subtract, reduce -> [128, 1] partials
  - host: loss = sum(partials) / B
"""

import os as _os
import sys

import numpy as np

if "/opt/trn_rl_repo" not in sys.path:
    sys.path.insert(0, "/opt/trn_rl_repo")

# a previously crashed run can leave a core wedged; reset at init is harmless
_os.environ.setdefault("NEURON_RT_RESET_CORES", "1")

B = 32768
C = 1000
NCORES = 8
RPC = B // NCORES          # rows per core = 4096
P = 128                    # partitions
NBLK = RPC // P            # 32 blocks of 128 rows per core
GPB = int(_os.environ.get("GPB", "16"))  # blocks per group tile
NG = NBLK // GPB           # groups
DPB = int(_os.environ.get("DPB", "8"))  # blocks per DMA (1 MB contiguous fp8)


def _mk_pattern(na, nd, np_):
    counts = {"A": na, "D": nd, "P": np_}
    counts = {k: v for k, v in counts.items() if v > 0}
    used = {k: 0 for k in counts}
    out = []
    for _ in range(sum(counts.values())):
        best = max(counts, key=lambda e: (counts[e] - used[e]) / counts[e])
        out.append(best)
        used[best] += 1
    return "".join(out)


BLOCK_PATTERN = _os.environ.get("BLOCK_PATTERN", "")
if not BLOCK_PATTERN:
    BLOCK_PATTERN = _mk_pattern(
        int(_os.environ.get("NA", "13")),
        int(_os.environ.get("ND", "0")),
        int(_os.environ.get("NP", "19")),
    )
assert len(BLOCK_PATTERN) == NBLK, BLOCK_PATTERN

# fold the two block halves with one 2x-mode tensor_tensor add before the
# 1x-mode accum pass (halves the elements the slow accumulator sees)
FOLD = _os.environ.get("FOLD", "0") == "1"

# emit one gpsimd convert over two adjacent 'P' blocks (halves Pool's
# instruction count and semaphore traffic)
PAIRP = _os.environ.get("PAIRP", "1") == "1"

# fp16 Schraudolph exp: bitcast16(round(A16*x + B16)) ~ exp(x). c calibrated
# so mean relative error over uniform mantissa positions is ~zero.
_SCHRAUDOLPH_C = 0.05640058203281112
A16 = float(np.float32(2**10 / np.log(2)))
B16 = float(np.float32((15 - _SCHRAUDOLPH_C) * 2**10))

# fp32 tail log via bitcast: ln(s) ~ (bitcast_i32(s)*2^-23 - (127 - c2)) * ln2
C2LOG = 0.0573049591429322
LOG_APPROX = _os.environ.get("LOG_APPROX", "1") == "1"
LG_A = float(np.float32(np.log(2) / 2**23))
LG_B = float(np.float32(-(127 - C2LOG) * np.log(2)))

_CACHE = {}


def build_nc(repeat=1, loop=1):
    import contextlib

    import concourse.bacc as bacc
    import concourse.tile as tile
    from concourse import mybir

    nc = bacc.Bacc("TRN2", target_bir_lowering=False, debug=False, num_devices=NCORES)

    x = nc.dram_tensor("x", [RPC, C], mybir.dt.float8e4, kind="ExternalInput").ap()
    out = nc.dram_tensor("out", [P, 1], mybir.dt.float32, kind="ExternalOutput").ap()

    # row (g*GPB + b)*128 + p  ->  group g, sbuf [p, b, c] (host pre-transpose)
    x_r = x.rearrange("(g p b) c -> g p b c", p=P, b=GPB)

    with tile.TileContext(nc) as tc:
        with (
            tc.tile_pool(name="xbig", bufs=2) as x_pool,
            tc.tile_pool(name="ebig", bufs=2) as e_pool,
            tc.tile_pool(name="small", bufs=1) as small,
        ):
            s_all = small.tile([P, NBLK], mybir.dt.float32)
            xv_all = small.tile([P, NBLK], mybir.dt.float32)
            junk_v = small.tile([P, C], mybir.dt.float16)
            ft = None
            if FOLD:
                ft = small.tile([P, C // 2], mybir.dt.float16)

            loop_cm = tc.For_i(0, loop, 1) if loop > 1 else contextlib.nullcontext()
            with loop_cm:
                for i, g in enumerate(
                    [g for _ in range(repeat) for g in range(NG)]
                ):
                    xt = x_pool.tile([P, GPB, C], mybir.dt.float8e4, tag="xt")
                    eg = e_pool.tile([P, GPB, C], mybir.dt.float16, tag="eg")
                    if i == 0:
                        # small leading chunks so compute starts sooner
                        splits = [0, 1, 2, 4]
                        while splits[-1] < GPB:
                            splits.append(min(splits[-1] + DPB, GPB))
                    else:
                        splits = list(range(0, GPB + 1, DPB))
                    for lo, hi in zip(splits[:-1], splits[1:]):
                        nc.sync.dma_start(
                            out=xt[:, lo:hi, :],
                            in_=x_r[g, :, lo:hi, :],
                        )

                    # label values: column 0 of each block, one op per group
                    nc.vector.tensor_scalar(
                        out=xv_all[:, g * GPB : (g + 1) * GPB],
                        in0=xt[:, :, 0:1].rearrange("p b c -> p (b c)"),
                        scalar1=1.0,
                        scalar2=0.0,
                        op0=mybir.AluOpType.mult,
                        op1=mybir.AluOpType.add,
                    )

                    conv_done = set()
                    for b in range(GPB):
                        k = g * GPB + b
                        kind = BLOCK_PATTERN[k % len(BLOCK_PATTERN)]
                        if (
                            PAIRP
                            and kind == "P"
                            and b not in conv_done
                            and b + 1 < GPB
                            and BLOCK_PATTERN[(k + 1) % len(BLOCK_PATTERN)] == "P"
                        ):
                            nc.gpsimd.tensor_scalar(
                                out=eg[:, b : b + 2, :].bitcast(mybir.dt.int16),
                                in0=xt[:, b : b + 2, :],
                                scalar1=A16,
                                scalar2=B16,
                                op0=mybir.AluOpType.mult,
                                op1=mybir.AluOpType.add,
                            )
                            conv_done.add(b)
                            conv_done.add(b + 1)
                        if kind == "A":
                            nc.scalar.activation(
                                out=eg[:, b, :],
                                in_=xt[:, b, :],
                                func=mybir.ActivationFunctionType.Exp,
                                accum_out=s_all[:, k : k + 1],
                            )
                        else:
                            if b not in conv_done:
                                conv = nc.vector if kind == "D" else nc.gpsimd
                                conv.tensor_scalar(
                                    out=eg[:, b, :].bitcast(mybir.dt.int16),
                                    in0=xt[:, b, :],
                                    scalar1=A16,
                                    scalar2=B16,
                                    op0=mybir.AluOpType.mult,
                                    op1=mybir.AluOpType.add,
                                )
                            if FOLD:
                                nc.vector.tensor_add(
                                    ft[:], eg[:, b, 0 : C // 2], eg[:, b, C // 2 : C]
                                )
                                src = ft[:]
                            else:
                                src = eg[:, b, :]
                            nc.vector.tensor_scalar(
                                out=junk_v[:, 0 : (C // 2 if FOLD else C)],
                                in0=src,
                                scalar1=1.0,
                                scalar2=0.0,
                                op0=mybir.AluOpType.mult,
                                op1=mybir.AluOpType.add,
                                accum_out=s_all[:, k : k + 1],
                            )

            lse = small.tile([P, NBLK], mybir.dt.float32)
            if LOG_APPROX:
                nc.vector.tensor_scalar(
                    out=lse[:],
                    in0=s_all[:].bitcast(mybir.dt.int32),
                    scalar1=LG_A,
                    scalar2=LG_B,
                    op0=mybir.AluOpType.mult,
                    op1=mybir.AluOpType.add,
                )
            else:
                nc.scalar.activation(
                    out=lse[:], in_=s_all[:], func=mybir.ActivationFunctionType.Ln
                )
            diff = small.tile([P, NBLK], mybir.dt.float32)
            nc.vector.tensor_sub(diff[:], lse[:], xv_all[:])
            final = small.tile([P, 1], mybir.dt.float32)
            nc.vector.tensor_reduce(
                out=final[:], in_=diff[:], axis=mybir.AxisListType.X,
                op=mybir.AluOpType.add,
            )
            nc.sync.dma_start(out=out, in_=final[:])

    nc.compile()
    return nc


def make_inputs(cls_score, label):
    """Host-side sharding: cast to fp8, swap each row's label column into
    column 0 (logsumexp is permutation-invariant, so only the label-value
    read changes), pre-transpose so each partition's group DMA is one
    contiguous run."""
    import ml_dtypes

    cls_score = np.asarray(cls_score, dtype=np.float32)
    label = np.asarray(label).astype(np.int64)
    assert cls_score.shape == (B, C), cls_score.shape
    assert label.shape == (B,), label.shape
    x8 = cls_score.astype(ml_dtypes.float8_e4m3)

    rows = np.arange(B)
    col0 = x8[rows, 0].copy()
    x8[rows, 0] = x8[rows, label]
    x8[rows, label] = col0

    in_maps = []
    for c in range(NCORES):
        xc = x8[c * RPC : (c + 1) * RPC]
        xc = (
            xc.reshape(NG, GPB, P, C)
            .transpose(0, 2, 1, 3)
            .reshape(RPC, C)
        )
        in_maps.append({"x": np.ascontiguousarray(xc)})
    return in_maps


def _run(cls_score, label, **spmd_kwargs):
    import time

    from concourse.bass_utils import run_bass_kernel_spmd

    if "nc" not in _CACHE:
        _CACHE["nc"] = build_nc()
    nc = _CACHE["nc"]

    in_maps = make_inputs(cls_score, label)
    last_err = None
    for attempt in range(4):
        try:
            res = run_bass_kernel_spmd(
                nc, in_maps, core_ids=list(range(NCORES)), **spmd_kwargs
            )
            break
        except Exception as e:  # transient device-unrecoverable states heal
            last_err = e
            time.sleep(10 * (attempt + 1))
    else:
        raise last_err
    total = np.float64(0.0)
    for r in res.results:
        total += r["out"].astype(np.float64).sum()
    return np.float32(total / B), res


def kernel(cls_score, label, xi=None, **_ignored):
    return _run(cls_score, label)[0]


if __name__ == "__main__":
    rng = np.random.default_rng(0)
    x = rng.standard_normal((B, C), dtype=np.float32)
    lab = rng.integers(0, C, size=(B,)).astype(np.int64)
    got = kernel(x, lab, np.ones((C, C), np.float32))
    m = x.max(axis=-1, keepdims=True)
    lse = (np.log(np.exp(x - m).sum(-1)) + m[:, 0]).astype(np.float64)
    want = (lse - x[np.arange(B), lab]).mean()
    print("kernel:", got, "ref:", want, "rel:", abs(got - want) / abs(want))

